# revision 30
# baseline (speedup 1.0000x reference)
"""Trainium2 Bass kernel for nn_ConvAttention (dwconv3x3->BN->GELU->1x1 conv
q/k/v branches, 8-head attention over 32x32 tokens, 1x1 out-proj, BN).

Sharding: data-parallel over batch B=8 across the 8 NeuronCores (one image
per core). Training-mode BatchNorm stats are computed exactly on the host
(cheap numpy recompute of the depthwise conv for statistics only); the final
BN is applied on the host after gathering.

Device per-core pipeline (all matmuls bf16):
  x -> pad -> bf16 -> 9 accumulated diagonal matmuls per 128-channel block
  (depthwise conv) -> fused scale/bias+GELU on ACT (folded BN, bf16 out) ->
  bf16 pointwise matmuls for q,k (M=112 head-pairs); v-branch pointwise is
  computed TRANSPOSED (lhsT=y_v chunk, rhs=pw_v^T) producing v^T tiles
  directly (no PE transposes). Attention per head-pair:
    phase A: S^T chunks = k^T q as 2 concurrent row-tiled (K=48->64) matmuls
             (heads at partitions 0-47 / 64-111), exp on ACT -> bf16 P^T
    phase B: O = [v|1]^T P^T accumulated over chunks as 2 concurrent
             col-tiled (M=49->64) matmuls; denom rows 48/112; reciprocal on
             DVE (bf16), ones-matmul broadcast, DVE multiply -> pair tile
             [112, N] with heads at rows 0-47 / 64-111 (rows 48-63 zeroed).
  Out-projection: K=112 matmuls per pair with zero-padded w rows 48-63.
"""

import sys

sys.path.insert(0, "/opt/trn_rl_repo")

import numpy as np
import ml_dtypes

import concourse.bass as bass
import concourse.mybir as mybir
import concourse.tile as tile
from concourse.bass_utils import run_bass_kernel_spmd

BF16 = ml_dtypes.bfloat16
F32 = mybir.dt.float32
BF = mybir.dt.bfloat16
F32R = mybir.dt.float32r
F16 = mybir.dt.float16

B, C, H, W = 8, 384, 32, 32
N = H * W
HEADS, HD = 8, 48
SCALE = float(HD ** -0.5)
NBLK = C // 128          # 3 channel blocks
NPAIR = HEADS // 2       # 4 head pairs
EPS = 1e-5

_GELU = mybir.ActivationFunctionType.Gelu
GELU_FUNC = [_GELU]
_EXP = mybir.ActivationFunctionType.Exp
_LN = mybir.ActivationFunctionType.Ln


# ---------------------------------------------------------------- wait split
def _split_excess_waits(nc, max_waits=1):
    """Old walrus rejects >1 sync wait per instruction; hoist extras onto
    NoOps inserted just before, on the same engine (queue order preserved)."""
    n = 0
    for f in nc.m.functions:
        for bb in f.blocks:
            out, changed = [], False
            for inst in bb.instructions:
                si = inst.sync_info
                waits = list(si.on_wait) if si is not None else []
                if len(waits) > max_waits:
                    excess, keep = waits[:-max_waits], waits[-max_waits:]
                    for j, w in enumerate(excess):
                        nop = mybir.InstNoOp(
                            name=f"WSPLIT-{inst.name}-{j}", ins=[], outs=[])
                        nop.engine = inst.engine
                        nop.sync_info = mybir.SyncInfo(on_wait=[w], on_update=[])
                        out.append(nop)
                        n += 1
                    inst.sync_info = mybir.SyncInfo(
                        on_wait=keep, on_update=list(si.on_update))
                    changed = True
                out.append(inst)
            if changed:
                bb.instructions = out
    return n


# ---------------------------------------------------------- ldweights dedupe
def _dedupe_ldweights(nc):
    """After scheduling, consecutive PE matmuls that use the identical
    stationary operand don't need to reload it: set ldweights=False on the
    followers. Any other PE instruction between them is a barrier."""
    def sig(inst):
        w = inst.ins[1]
        try:
            return (str(w), inst.perf_mode, inst.is_transpose,
                    tuple(inst.tile_position or ()))
        except Exception:
            return None
    n = 0
    for f in nc.m.functions:
        for bb in f.blocks:
            last = {}
            for inst in bb.instructions:
                eng = inst.engine
                if not isinstance(inst, mybir.InstMatmult):
                    if eng in last and not isinstance(inst, mybir.InstNoOp):
                        last.pop(eng, None)
                    continue
                s = sig(inst)
                if s is not None and last.get(eng) == s:
                    inst.ldweights = False
                    n += 1
                else:
                    last[eng] = s
    return n


# ---------------------------------------------------------------- builder
def build_kernel(split_waits=True):
    nc = bass.Bass("TRN2", target_bir_lowering=False, debug=False)

    x_d = nc.dram_tensor("x", [C, H, W], F32, kind="ExternalInput").ap()
    diag_d = nc.dram_tensor("diags", [3, NBLK, 128, 9, 128], BF,
                            kind="ExternalInput").ap()
    # A/D packed: [128, br*blk*2] (col = 2*(3*br+blk) + {0:A,1:D})
    AD_d = nc.dram_tensor("scaleAD", [128, 18], F32,
                          kind="ExternalInput").ap()
    # q,k pointwise: [2, kc, 128, pair, 112] bf16 (head0 cols 0-47, head1 64-111)
    pwqkT_d = nc.dram_tensor("pwqkT", [2, NBLK, 128, NPAIR, 112], BF,
                             kind="ExternalInput").ap()
    # v pointwise transposed-producing: [kc, 128, pair, 98] bf16
    # cols 0-47 = head0 dims, col 48 = 0, cols 49-96 = head1 dims, col 97 = 0
    pwvT_d = nc.dram_tensor("pwvT", [NBLK, 128, NPAIR, 98], BF,
                            kind="ExternalInput").ap()
    # out-proj per pair: [pair, 128, C] bf16; rows 0,49-64,113-127 ZERO,
    # rows 1-48 / 65-112 = the two heads' weights (denom-first O layout)
    woPairT_d = nc.dram_tensor("woPairT", [NPAIR, 128, C], F32R,
                               kind="ExternalInput").ap()
    ones_d = nc.dram_tensor("onesc", [65, 64], F32,
                            kind="ExternalInput").ap()
    out_d = nc.dram_tensor("out", [C, N], F32, kind="ExternalOutput").ap()

    with tile.TileContext(nc) as tc:
        from contextlib import ExitStack
        ctx = ExitStack()
        with ctx:
            cpool = ctx.enter_context(tc.tile_pool(name="consts", bufs=1))
            xpool = ctx.enter_context(tc.tile_pool(name="xin", bufs=2))
            padpool = ctx.enter_context(tc.tile_pool(name="pads", bufs=1))
            yhpool = ctx.enter_context(tc.tile_pool(name="yh", bufs=1))
            qkpool = ctx.enter_context(tc.tile_pool(name="qk", bufs=1))
            vtpool = ctx.enter_context(tc.tile_pool(name="vt", bufs=1))
            ptpool = ctx.enter_context(tc.tile_pool(name="pt", bufs=26))
            osbpool = ctx.enter_context(tc.tile_pool(name="osb", bufs=1))
            rpool = ctx.enter_context(tc.tile_pool(name="rin", bufs=2))
            outpool = ctx.enter_context(tc.tile_pool(name="outsb", bufs=2))

            # one shared rotating PSUM tag: 4 slots x [128,1024]f32 = 8 banks
            pspool = ctx.enter_context(
                tc.tile_pool(name="ps", bufs=4, space="PSUM"))

            # ---------------- constants
            # all-ones lhsT rows for the denominator broadcast (rows 0 and
            # 64 used, matching each col-tile's base partition)
            ones_f = cpool.tile([65, 64], F32, tag="ones")
            nc.sync.dma_start(ones_f[:], ones_d[:])

            xpad = {}
            for blk in range(NBLK):
                xt = xpool.tile([128, H, W], F32)
                nc.sync.dma_start(xt[:], x_d[blk * 128:(blk + 1) * 128])
                xp = padpool.tile([128, H + 2, W + 2], BF, tag=f"xpad{blk}")
                nc.gpsimd.memset(xp[:], 0.0)
                nc.vector.tensor_copy(xp[:, 1:H + 1, 1:W + 1], xt[:])
                xpad[blk] = xp

            diag_t = {}
            for br in range(3):
                for blk in range(NBLK):
                    t = cpool.tile([128, 9, 128], BF, tag=f"diag{br}_{blk}")
                    nc.sync.dma_start(t[:], diag_d[br, blk])
                    diag_t[(br, blk)] = t
            AD_t = cpool.tile([128, 18], F32, tag="AD")
            nc.sync.dma_start(AD_t[:], AD_d[:])
            pwqk_t = {}
            for br in range(2):
                for kc in range(NBLK):
                    t = cpool.tile([128, NPAIR, 112], BF, tag=f"pwqk{br}_{kc}")
                    nc.sync.dma_start(t[:], pwqkT_d[br, kc])
                    pwqk_t[(br, kc)] = t
            pwv_t = {}
            for kc in range(NBLK):
                t = cpool.tile([128, NPAIR, 98], BF, tag=f"pwv{kc}")
                nc.sync.dma_start(t[:], pwvT_d[kc])
                pwv_t[kc] = t
            wo_t = {}
            for pair in range(NPAIR):
                t = cpool.tile([128, C], F32R, tag=f"wo{pair}")
                nc.sync.dma_start(t[:], woPairT_d[pair])
                wo_t[pair] = t

            # osb pair tiles: rows 0-47 = head0 O, 64-111 = head1 O,
            # rows 48-63 zeroed once (K=112 outproj sees w-rows 48-63 = 0,
            # but 0*NaN = NaN so the garbage rows must be cleared).
            osb_t = {}
            for pair in range(NPAIR):
                t = osbpool.tile([128, N], F32R, tag=f"osb{pair}")
                osb_t[pair] = t

            # ---------------- depthwise conv + BN + GELU (q,k first, v later)
            yh_t = {}

            def conv_branch_blk(br, blk):
                py = pspool.tile([128, N], F32, tag="ps")
                for tap in range(9):
                    di, dj = tap // 3, tap % 3
                    for hf in range(2):
                        nc.tensor.matmul(
                            py[:, hf * 512:(hf + 1) * 512],
                            diag_t[(br, blk)][:, tap, :],
                            xpad[blk][:, di + 16 * hf:di + 16 * hf + 16,
                                      dj:dj + W],
                            start=(tap == 0), stop=(tap == 8))
                yh = yhpool.tile([128, N], BF, tag=f"yh{br}_{blk}")
                col = 2 * (3 * br + blk)
                nc.scalar.activation(
                    yh[:], py[:], GELU_FUNC[0],
                    bias=AD_t[:, col + 1:col + 2],
                    scale=AD_t[:, col:col + 1])
                yh_t[(br, blk)] = yh

            for br in range(2):
                for blk in range(NBLK):
                    conv_branch_blk(br, blk)

            # ---------------- pointwise q,k (bf16, M=112 head-pairs)
            qk_sb = {}

            def pw_qk(pair):
                for br in range(2):
                    pp = pspool.tile([112, N], F32, tag="ps")
                    for kc in range(NBLK):
                        lhsT = pwqk_t[(br, kc)][:, pair, :]
                        for nch in range(2):
                            nc.tensor.matmul(
                                pp[:, nch * 512:(nch + 1) * 512],
                                lhsT,
                                yh_t[(br, kc)][:, nch * 512:(nch + 1) * 512],
                                start=(kc == 0), stop=(kc == NBLK - 1))
                    sb = qkpool.tile([112, N], BF, tag=f"qk{br}_{pair}")
                    nc.vector.tensor_copy(sb[:], pp[:])
                    qk_sb[(br, pair)] = sb

            vt_t = {}

            def make_vt(js):
                # all 4 pairs in one 392-col matmul per (j, kc)
                for j in js:
                    pv = pspool.tile([128, NPAIR * 98], F32, tag="ps",
                                     name=f"pv{j}")
                    for kc in range(NBLK):
                        nc.tensor.matmul(
                            pv[:],
                            yh_t[(2, kc)][:, j * 128:(j + 1) * 128],
                            pwv_t[kc][:, :, :],
                            start=(kc == 0), stop=(kc == NBLK - 1))
                    vt = vtpool.tile([128, NPAIR, 98], BF, tag=f"vt{j}",
                                     name=f"vt{j}")
                    nc.vector.tensor_copy(vt[:], pv[:])
                    nc.gpsimd.memset(vt[:, :, 0:1], 1.0)
                    nc.gpsimd.memset(vt[:, :, 49:50], 1.0)
                    vt_t[j] = vt

            # ---------------- attention per pair (A: S+exp, B: O+divide)
            pts_all = {}

            def attn_A(pair, js):
                q_sb = qk_sb[(0, pair)]
                k_sb = qk_sb[(1, pair)]
                pts = pts_all.setdefault(pair, {})
                for j in js:
                    pS = {}
                    for hh in range(2):
                        off = 64 * hh
                        pS[hh] = pspool.tile([128, N], F32, tag="ps",
                                             name=f"pS{pair}_{j}_{hh}")
                        for nch in range(2):
                            nc.tensor.matmul(
                                pS[hh][:, nch * 512:(nch + 1) * 512],
                                k_sb[off:off + 48, j * 128:(j + 1) * 128],
                                q_sb[off:off + 48,
                                     nch * 512:(nch + 1) * 512],
                                start=True, stop=True)
                    for hh in range(2):
                        pt = ptpool.tile([128, N], BF, tag="pt")
                        nc.scalar.activation(
                            pt[:], pS[hh][:], _EXP, bias=0.0, scale=SCALE)
                        pts[(hh, j)] = pt

            def attn_B(pair):
                pts = pts_all[pair]
                pO = pspool.tile([128, N], F32, tag="ps")
                # zero the junk bands (rows 49-63 / 113-127; the 32-aligned
                # memset also covers rows overwritten by the matmuls below)
                nc.vector.memset(pO[32:64, :], 0.0)
                nc.vector.memset(pO[96:128, :], 0.0)
                for j in range(8):
                    for hh in range(2):
                        lhsT = vt_t[j][:, pair, 49 * hh:49 * hh + 49]
                        for nch in range(2):
                            nc.tensor.matmul(
                                pO[64 * hh:64 * hh + 49,
                                   nch * 512:(nch + 1) * 512],
                                lhsT,
                                pts[(hh, j)][:, nch * 512:(nch + 1) * 512],
                                start=(j == 0), stop=(j == 7))
                # divide by the denominators (rows 0 / 64 of pO, since the
                # ones column is FIRST in vt): reciprocal on DVE at aligned
                # bases, broadcast to 64-row bands via ones-matmul, multiply.
                # the 1/denom -> O chain must stay fp32: O has a large
                # common-mode component (diffuse softmax ~ mean of v) and
                # the final BN divides by the small per-channel variance,
                # amplifying any coherent bf16 jitter ~5x over tolerance.
                # O[:, n] /= r[n] via exp(-ln r) (all fp32: O has a large
                # common-mode component and the final BN amplifies coherent
                # error, so no bf16 anywhere in this chain): ACT ln of the
                # denom rows, -1s-matmul broadcast, ACT exp PSUM->SBUF,
                # one DVE multiply per 64-row band.
                pb = pspool.tile([128, N], F32, tag="ps")
                lnr0 = rpool.tile([1, N], F32, tag="lnr0",
                                  name=f"lnr0_{pair}")
                lnr1 = rpool.tile([65, N], F32, tag="lnr1",
                                  name=f"lnr1_{pair}")
                nc.scalar.activation(lnr0[0:1, :], pO[0:1, :], _LN,
                                     bias=0.0, scale=1.0)
                nc.scalar.activation(lnr1[64:65, :], pO[64:65, :], _LN,
                                     bias=0.0, scale=1.0)
                for nch in range(2):
                    nc.tensor.matmul(
                        pb[0:64, nch * 512:(nch + 1) * 512],
                        ones_f[0:1, :],
                        lnr0[0:1, nch * 512:(nch + 1) * 512],
                        start=True, stop=True)
                    nc.tensor.matmul(
                        pb[64:128, nch * 512:(nch + 1) * 512],
                        ones_f[64:65, :],
                        lnr1[64:65, nch * 512:(nch + 1) * 512],
                        start=True, stop=True)
                bc = rpool.tile([128, N], F32, tag="bc", name=f"bc{pair}")
                nc.scalar.activation(bc[:], pb[:], _EXP, bias=0.0, scale=1.0)
                for hh in range(2):
                    off = 64 * hh
                    nc.vector.tensor_mul(
                        osb_t[pair][off:off + 64, :],
                        pO[off:off + 64, :], bc[off:off + 64, :])

            # Start attention ASAP (pair0 q/k pointwise, then its S/exp
            # stream). Emission order == PSUM slot rotation order, so the
            # v-branch conv / vT production / remaining pointwise are
            # interleaved between S chunks at the granularity their PSUM
            # slots free up, filling the PE's exp-wait bubbles.
            pw_qk(0)
            conv_branch_blk(2, 0)
            conv_branch_blk(2, 1)
            conv_branch_blk(2, 2)
            attn_A(0, [0, 1, 2, 3])
            make_vt([0, 1, 2, 3])
            pw_qk(1)
            attn_A(0, [4, 5, 6, 7])
            make_vt([4, 5, 6, 7])
            attn_A(1, [0, 1, 2, 3])
            attn_B(0)
            pw_qk(2)
            attn_A(1, [4, 5, 6, 7])
            attn_A(2, [0, 1, 2, 3])
            attn_B(1)
            pw_qk(3)
            attn_A(2, [4, 5, 6, 7])
            attn_A(3, [0, 1, 2, 3])
            attn_B(2)
            attn_A(3, [4, 5, 6, 7])
            attn_B(3)

            # ---------------- out projection (K=112 per pair, zero rows 48-63)
            for m in range(NBLK):
                po = pspool.tile([128, N], F32, tag="ps")
                for pair in range(NPAIR):
                    lhsT = wo_t[pair][:, m * 128:(m + 1) * 128]
                    for nch in range(2):
                        nc.tensor.matmul(
                            po[:, nch * 512:(nch + 1) * 512],
                            lhsT,
                            osb_t[pair][:, nch * 512:(nch + 1) * 512],
                            start=(pair == 0), stop=(pair == NPAIR - 1))
                ob = outpool.tile([128, N], F32)
                nc.vector.tensor_copy(ob[:], po[:])
                nc.sync.dma_start(out_d[m * 128:(m + 1) * 128, :], ob[:])

    if split_waits:
        _split_excess_waits(nc)
    return nc


_NC_CACHE = {}


def _get_nc():
    if "nc" not in _NC_CACHE:
        _NC_CACHE["nc"] = build_kernel()
    return _NC_CACHE["nc"]


# ---------------------------------------------------------------- host prep
def _conv_dw_np(x, dw):
    Bx, Cx, Hx, Wx = x.shape
    xp = np.zeros((Bx, Cx, Hx + 2, Wx + 2), np.float32)
    xp[:, :, 1:Hx + 1, 1:Wx + 1] = x
    y = np.zeros((Bx, Cx, Hx, Wx), np.float32)
    for i in range(3):
        for j in range(3):
            y += dw[None, :, i, j, None, None] * \
                xp[:, :, i:i + Hx, j:j + Wx]
    return y


def _host_prep(inputs):
    x = np.ascontiguousarray(np.asarray(inputs["x"], np.float32))
    diags = np.zeros((3, NBLK, 128, 9, 128), BF16)
    AD = np.zeros((128, 18), np.float32)
    pwqkT = np.zeros((2, NBLK, 128, NPAIR, 112), BF16)
    pwvT = np.zeros((NBLK, 128, NPAIR, 98), BF16)
    woPairT = np.zeros((NPAIR, 128, C), np.float32)
    idx = np.arange(128)
    for br, p in enumerate(["q", "k", "v"]):
        dw = np.asarray(inputs[f"dw_{p}"], np.float32).reshape(C, 3, 3)
        dwb = dw.astype(BF16).astype(np.float32)
        y = _conv_dw_np(x, dwb)          # matches device conv (bf16 weights)
        m = y.astype(np.float64).mean(axis=(0, 2, 3))
        v = y.astype(np.float64).var(axis=(0, 2, 3))
        g = np.asarray(inputs[f"g_{p}"], np.float64)
        bb = np.asarray(inputs[f"b_{p}"], np.float64)
        a = g / np.sqrt(v + EPS)
        dd = (bb - m * a)
        for blk in range(NBLK):
            col = 2 * (3 * br + blk)
            AD[:, col] = a[blk * 128:(blk + 1) * 128].astype(np.float32)
            AD[:, col + 1] = dd[blk * 128:(blk + 1) * 128].astype(np.float32)
            for tap in range(9):
                diags[br, blk, idx, tap, idx] = \
                    dwb[blk * 128:(blk + 1) * 128, tap // 3, tap % 3]
        pwt = np.asarray(inputs[f"pw_{p}"], np.float32).T  # (c_in, c_out)
        for kc in range(NBLK):
            pin = pwt[kc * 128:(kc + 1) * 128]  # (128, C_out)
            for pair in range(NPAIR):
                h0 = pin[:, (2 * pair) * 48:(2 * pair + 1) * 48]
                h1 = pin[:, (2 * pair + 1) * 48:(2 * pair + 2) * 48]
                if br < 2:
                    pwqkT[br, kc, :, pair, 0:48] = h0
                    pwqkT[br, kc, :, pair, 64:112] = h1
                else:
                    pwvT[kc, :, pair, 1:49] = h0
                    pwvT[kc, :, pair, 50:98] = h1
    w_out = np.asarray(inputs["w_out"], np.float32)  # (C_out, C_in)
    for pair in range(NPAIR):
        woPairT[pair, 1:49, :] = w_out[:, (2 * pair) * 48:
                                       (2 * pair + 1) * 48].T
        woPairT[pair, 65:113, :] = w_out[:, (2 * pair + 1) * 48:
                                         (2 * pair + 2) * 48].T
    return x, diags, AD, pwqkT, pwvT, woPairT


def kernel(**inputs) -> np.ndarray:
    x, diags, AD, pwqkT, pwvT, woPairT = _host_prep(inputs)
    nc = _get_nc()
    in_maps = []
    for b in range(B):
        in_maps.append({
            "x": np.ascontiguousarray(x[b]),
            "diags": diags,
            "scaleAD": AD,
            "pwqkT": pwqkT,
            "pwvT": pwvT,
            "woPairT": woPairT,
            "onesc": np.full((65, 64), -1.0, np.float32),
        })
    res = run_bass_kernel_spmd(nc, in_maps, list(range(B)))
    out = np.stack([res.results[b]["out"] for b in range(B)])  # (B, C, N)

    o64 = out.astype(np.float64)
    m = o64.mean(axis=(0, 2))
    v = o64.var(axis=(0, 2))
    g = np.asarray(inputs["g_out"], np.float64)
    bb = np.asarray(inputs["b_out"], np.float64)
    res_f = (o64 - m[None, :, None]) / np.sqrt(v + EPS)[None, :, None] * \
        g[None, :, None] + bb[None, :, None]
    return res_f.reshape(B, C, H, W).astype(np.float32)
